# revision 6
# baseline (speedup 1.0000x reference)
"""Multi-head causal self-attention with RoPE on 8 Trainium2 NeuronCores.

Problem: x[2,2048,2048], wq/wk/wv/wo[2048,2048] fp32, 16 heads (hd=128),
interleaved RoPE, causal softmax, Megatron-style tensor parallelism over
heads: 2 heads per core, wo row-sharded, partial outputs summed on host.

All matmuls run as float32r (fp32 rounded to 11-bit mantissa; full PE rate
at moving-dim >= 256). Host pre-rounds DRAM inputs to f32r; on-device
producers write f32r directly.

Layout strategy (per core, per batch b):
  - host supplies xT = x^T [d, s] (f32r) and weight slices pre-transposed
  - projections: qT,kT per head via lhsT=w-tile [d,e], rhs=xT [d,s512]
    -> q^T,k^T [e=128, s] directly; v natural [s, e] via lhsT=xT-subtile
  - RoPE: qrotT = RotL.T @ qT (signed pair-swap as a matmul), then
    q_roped = qT*cosT + qrotT*sinT on DVE (tables indexed [e, s])
  - attention per (b, h), per q-block of 512:
      scoresT[kv=128, q=512] = kT-tile.T @ qT-block   (one matmul, d=128)
      attn = exp(scoresT) on ACT (-> f32r), staircase tiles masked (DVE mul)
      oT[d, q] += v-tile.T @ attn ; rowsumB[128, q] += ones.T @ attn
      oT_norm = oT * reciprocal(rowsumB)  (-> f32r)
  - output projection: yT[e, s] = sum_ct woT-tile.T @ oT  -> DRAM
  - host: y = sum over cores of yT^T
"""

import os
import sys

for _p in ("/opt/trn_rl_repo", "/root/.axon_site/_ro/trn_rl_repo"):
    if os.path.isdir(_p) and _p not in sys.path:
        sys.path.append(_p)

import numpy as np

import concourse.bacc as bacc
import concourse.mybir as mybir
import concourse.tile as tile
from concourse.alu_op_type import AluOpType
from concourse.bass_utils import run_bass_kernel_spmd

F32 = mybir.dt.float32
F32R = mybir.dt.float32r
BF16 = mybir.dt.bfloat16

B, S, D = 2, 2048, 2048
H, HD = 16, 128
NCORES = 8
HPC = H // NCORES            # heads per core = 2
CPC = HPC * HD               # channels per core = 256
P = 128
SC = 512                     # s-chunk for projections / q-block for attention
NSC = S // SC                # 4
NDT = D // P                 # 16 contraction tiles
ROPE_THETA = 10000.0

Exp = mybir.ActivationFunctionType.Exp

last_exec_time_ns = None
_nc_cache = None


def _round_f32r(x):
    u = np.ascontiguousarray(x, dtype=np.float32).view(np.uint32)
    r = (u + np.uint32(0x7FF) + ((u >> np.uint32(12)) & np.uint32(1))) \
        & np.uint32(0xFFFFF000)
    return r.view(np.float32)


def _build_nc():
    nc = bacc.Bacc("TRN2", target_bir_lowering=False, debug=False)

    xT = nc.dram_tensor("xT", [B, D, S], F32R, kind="ExternalInput")
    wqkvT = nc.dram_tensor("wqkvT", [D, 6 * P], F32R, kind="ExternalInput")
    woT = nc.dram_tensor("woT", [CPC, D], F32R, kind="ExternalInput")
    cosT = nc.dram_tensor("cosT", [HD, S], F32, kind="ExternalInput")
    sinT = nc.dram_tensor("sinT", [HD, S], F32, kind="ExternalInput")
    rotL = nc.dram_tensor("rotL", [HD, HD], F32R, kind="ExternalInput")
    masks = nc.dram_tensor("masks", [4, P, SC], BF16, kind="ExternalInput")
    ones = nc.dram_tensor("ones", [P, P], F32R, kind="ExternalInput")
    yT = nc.dram_tensor("yT", [B, D, S], F32, kind="ExternalOutput")

    with tile.TileContext(nc) as tc:
        with tc.tile_pool(name="const", bufs=1) as constp, \
             tc.tile_pool(name="xp", bufs=17) as xp, \
             tc.tile_pool(name="qk", bufs=1) as qkp, \
             tc.tile_pool(name="vp", bufs=1) as vp, \
             tc.tile_pool(name="op", bufs=1) as op_, \
             tc.tile_pool(name="attn", bufs=3) as attnp, \
             tc.tile_pool(name="tmp", bufs=2) as tmpp, \
             tc.tile_pool(name="yt", bufs=2) as ytp, \
             tc.tile_pool(name="ps", bufs=3, space="PSUM") as psp, \
             tc.tile_pool(name="acc", bufs=2, space="PSUM") as accp:

            # ---- constants ----
            wq_sb = constp.tile([P, NDT, 6 * P], F32R)
            nc.sync.dma_start(wq_sb[:], wqkvT.rearrange("(o p) e -> p o e", p=P))
            wo_sb = constp.tile([P, CPC // P, D], F32R)
            nc.sync.dma_start(wo_sb[:], woT.rearrange("(o p) e -> p o e", p=P))
            cos_sb = constp.tile([P, S], F32)
            nc.sync.dma_start(cos_sb[:], cosT[:])
            sin_sb = constp.tile([P, S], F32)
            nc.sync.dma_start(sin_sb[:], sinT[:])
            rot_sb = constp.tile([P, P], F32R)
            nc.sync.dma_start(rot_sb[:], rotL[:])
            mask_sb = constp.tile([P, 4, SC], BF16)
            nc.sync.dma_start(mask_sb[:], masks.rearrange("m p q -> p m q"))
            ones_sb = constp.tile([P, P], F32R)
            nc.sync.dma_start(ones_sb[:], ones[:])

            for b in range(B):
                # ---- projections ----
                # qkT[e] for e in {q_h0, q_h1, k_h0, k_h1}: [128, S] transposed
                qkT = [qkp.tile([P, S], F32R, tag=f"qk{e}", name=f"qkT{e}")
                       for e in range(4)]
                # v natural [s_in=128, s_out=16, ch=256]
                v_sb = vp.tile([P, NDT, CPC], F32R, tag="v")
                for sc in range(NSC):
                    xts = []
                    for dt in range(NDT):
                        xt = xp.tile([P, SC], F32R, tag="xt")
                        nc.sync.dma_start(
                            xt[:], xT[b, dt * P:(dt + 1) * P, sc * SC:(sc + 1) * SC])
                        xts.append(xt)
                    for e in range(4):
                        pq = accp.tile([P, SC], F32, tag="acc")
                        for dt in range(NDT):
                            nc.tensor.matmul(pq[:],
                                             wq_sb[:, dt, e * P:(e + 1) * P],
                                             xts[dt][:],
                                             start=(dt == 0), stop=(dt == NDT - 1))
                        nc.scalar.copy(qkT[e][:, sc * SC:(sc + 1) * SC], pq[:])
                    for ss in range(SC // P):
                        pv = accp.tile([P, SC], F32, tag="acc")
                        pvv = pv[:, :CPC]
                        for dt in range(NDT):
                            nc.tensor.matmul(pvv,
                                             xts[dt][:, ss * P:(ss + 1) * P],
                                             wq_sb[:, dt, 4 * P:6 * P],
                                             start=(dt == 0), stop=(dt == NDT - 1))
                        nc.scalar.copy(v_sb[:, sc * (SC // P) + ss, :], pvv)

                # ---- RoPE on q,k (all 4 [128, S] tensors) ----
                for e in range(4):
                    for sc in range(NSC):
                        sl = slice(sc * SC, (sc + 1) * SC)
                        pr = psp.tile([P, SC], F32, tag="ps")
                        nc.tensor.matmul(pr[:], rot_sb[:], qkT[e][:, sl],
                                         start=True, stop=True)
                        tmp = tmpp.tile([P, SC], F32, tag="ropetmp")
                        nc.vector.tensor_tensor(tmp[:], pr[:], sin_sb[:, sl],
                                                AluOpType.mult)
                        nc.vector.tensor_tensor(qkT[e][:, sl], qkT[e][:, sl],
                                                cos_sb[:, sl], AluOpType.mult)
                        nc.vector.tensor_tensor(qkT[e][:, sl], qkT[e][:, sl],
                                                tmp[:], AluOpType.add)

                # ---- attention per head ----
                oT = op_.tile([P, HPC, S], F32R, tag="o")
                for h in range(HPC):
                    qTh, kTh = qkT[h], qkT[2 + h]
                    for j in range(NSC):
                        jsl = slice(j * SC, (j + 1) * SC)
                        n_kv = (SC // P) * (j + 1)
                        po = accp.tile([P, SC], F32, tag="acc")
                        prs = accp.tile([P, SC], F32, tag="acc")
                        for t in range(n_kv):
                            pscore = psp.tile([P, SC], F32, tag="ps")
                            nc.tensor.matmul(pscore[:],
                                             kTh[:, t * P:(t + 1) * P],
                                             qTh[:, jsl],
                                             start=True, stop=True)
                            attn = attnp.tile([P, SC], F32R, tag="attn")
                            nc.scalar.activation(attn[:], pscore[:], Exp,
                                                 bias=0.0, scale=1.0)
                            dp = t - (SC // P) * j
                            if dp >= 0:  # staircase tile: apply causal mask
                                nc.vector.tensor_tensor(attn[:], attn[:],
                                                        mask_sb[:, dp, :],
                                                        AluOpType.mult)
                            nc.tensor.matmul(po[:],
                                             v_sb[:, t, h * HD:(h + 1) * HD],
                                             attn[:],
                                             start=(t == 0), stop=(t == n_kv - 1))
                            nc.tensor.matmul(prs[:], ones_sb[:], attn[:],
                                             start=(t == 0), stop=(t == n_kv - 1))
                        recip = tmpp.tile([P, SC], F32, tag="recip")
                        nc.vector.reciprocal(recip[:], prs[:])
                        nc.vector.tensor_tensor(oT[:, h, jsl], po[:], recip[:],
                                                AluOpType.mult)

                # ---- output projection: yT[e,s] = sum_ct woT.T @ oT ----
                for et in range(NDT):
                    for sc in range(NSC):
                        py = accp.tile([P, SC], F32, tag="acc")
                        for ct in range(HPC):
                            nc.tensor.matmul(py[:],
                                             wo_sb[:, ct, et * P:(et + 1) * P],
                                             oT[:, ct, sc * SC:(sc + 1) * SC],
                                             start=(ct == 0), stop=(ct == HPC - 1))
                        yt = ytp.tile([P, SC], F32, tag="yt")
                        nc.scalar.copy(yt[:], py[:])
                        nc.sync.dma_start(
                            yT[b, et * P:(et + 1) * P, sc * SC:(sc + 1) * SC],
                            yt[:])
    nc.finalize()
    return nc


def _host_inputs(x, wq, wk, wv, wo):
    """Build per-core input maps (host-side shard + transform)."""
    scale = 1.0 / np.sqrt(np.float32(HD))

    xTr = _round_f32r(np.ascontiguousarray(x.transpose(0, 2, 1)))

    # RoPE tables in [e, s] layout (same for every head)
    inv_freq = 1.0 / (ROPE_THETA ** (np.arange(0, HD, 2, dtype=np.float64) / HD))
    ang = np.arange(S, dtype=np.float64)[None, :] * inv_freq[:, None]  # [64, S]
    cosT = np.repeat(np.cos(ang), 2, axis=0).astype(np.float32)  # [128, S]
    sinT = np.repeat(np.sin(ang), 2, axis=0).astype(np.float32)

    # signed pair-swap: qrot[2i] = -q[2i+1], qrot[2i+1] = q[2i]
    # matmul computes qrot[m, s] = sum_k rotL[k, m] q[k, s]
    rotL = np.zeros((HD, HD), dtype=np.float32)
    for i in range(HD // 2):
        rotL[2 * i + 1, 2 * i] = -1.0
        rotL[2 * i, 2 * i + 1] = 1.0

    import ml_dtypes
    r = np.arange(P)[:, None]
    c = np.arange(SC)[None, :]
    masks = np.stack([(c >= p * P + r) for p in range(4)]) \
        .astype(ml_dtypes.bfloat16)  # [4, 128, 512]

    wq_s = _round_f32r(wq * scale)
    wk_s = _round_f32r(wk)
    wv_s = _round_f32r(wv)
    wo_s = _round_f32r(wo)

    in_maps = []
    for cix in range(NCORES):
        rows = slice(cix * CPC, (cix + 1) * CPC)  # head-channel rows
        blocks = []
        for h in range(HPC):
            hr = slice((cix * HPC + h) * HD, (cix * HPC + h + 1) * HD)
            blocks.append(wq_s[hr])   # q_h: [128, D]
        for h in range(HPC):
            hr = slice((cix * HPC + h) * HD, (cix * HPC + h + 1) * HD)
            blocks.append(wk_s[hr])
        blocks.append(wv_s[rows])     # v both heads: [256, D]
        wqkvT = np.ascontiguousarray(
            np.concatenate(blocks, axis=0).T)  # [D, 768]
        woT = np.ascontiguousarray(wo_s[:, rows].T)  # [256, D]
        in_maps.append({
            "xT": xTr,
            "wqkvT": wqkvT,
            "woT": woT,
            "cosT": cosT,
            "sinT": sinT,
            "rotL": rotL,
            "masks": masks,
            "ones": np.ones((P, P), dtype=np.float32),
        })
    return in_maps


def _get_nc():
    global _nc_cache
    if _nc_cache is None:
        _nc_cache = _build_nc()
    return _nc_cache


def kernel(x, wq, wk, wv, wo, _trace=False):
    global last_exec_time_ns
    nc = _get_nc()
    in_maps = _host_inputs(np.asarray(x, dtype=np.float32),
                           np.asarray(wq, dtype=np.float32),
                           np.asarray(wk, dtype=np.float32),
                           np.asarray(wv, dtype=np.float32),
                           np.asarray(wo, dtype=np.float32))
    res = run_bass_kernel_spmd(nc, in_maps, core_ids=list(range(NCORES)),
                               trace=_trace)
    last_exec_time_ns = res.exec_time_ns
    y = np.zeros((B, S, D), dtype=np.float64)
    for cix in range(NCORES):
        y += res.results[cix]["yT"].transpose(0, 2, 1).astype(np.float64)
    return y.astype(np.float32)


# revision 7
# speedup vs baseline: 1.2028x; 1.2028x over previous
"""Multi-head causal self-attention with RoPE on 8 Trainium2 NeuronCores.

Problem: x[2,2048,2048], wq/wk/wv/wo[2048,2048] fp32, 16 heads (hd=128),
interleaved RoPE, causal softmax, Megatron-style tensor parallelism over
heads: 2 heads per core, wo row-sharded, partial outputs summed on host.

All matmuls run as float32r (fp32 rounded to 11-bit mantissa; full PE rate
at moving-dim >= 256). Host pre-rounds DRAM inputs to f32r; on-device
producers write f32r directly.

Layout strategy (per core, per batch b):
  - host supplies xT = x^T [d, s] (f32r) and weight slices pre-transposed
  - projections: qT,kT per head via lhsT=w-tile [d,e], rhs=xT [d,s512]
    -> q^T,k^T [e=128, s] directly; v natural [s, e] via lhsT=xT-subtile
  - RoPE: qrotT = RotL.T @ qT (signed pair-swap as a matmul), then
    q_roped = qT*cosT + qrotT*sinT on DVE (tables indexed [e, s])
  - attention per (b, h), per q-block of 512:
      scoresT[kv=128, q=512] = kT-tile.T @ qT-block   (one matmul, d=128)
      attn = exp(scoresT) on ACT (-> f32r), staircase tiles masked (DVE mul)
      oT[d, q] += v-tile.T @ attn ; rowsumB[128, q] += ones.T @ attn
      oT_norm = oT * reciprocal(rowsumB)  (-> f32r)
  - output projection: yT[e, s] = sum_ct woT-tile.T @ oT  -> DRAM
  - host: y = sum over cores of yT^T
"""

import os
import sys

for _p in ("/opt/trn_rl_repo", "/root/.axon_site/_ro/trn_rl_repo"):
    if os.path.isdir(_p) and _p not in sys.path:
        sys.path.append(_p)

import numpy as np

import concourse.bacc as bacc
import concourse.mybir as mybir
import concourse.tile as tile
from concourse.alu_op_type import AluOpType
from concourse.bass_utils import run_bass_kernel_spmd

F32 = mybir.dt.float32
F32R = mybir.dt.float32r
BF16 = mybir.dt.bfloat16

B, S, D = 2, 2048, 2048
H, HD = 16, 128
NCORES = 8
HPC = H // NCORES            # heads per core = 2
CPC = HPC * HD               # channels per core = 256
P = 128
SC = 512                     # s-chunk for projections / q-block for attention
NSC = S // SC                # 4
NDT = D // P                 # 16 contraction tiles
ROPE_THETA = 10000.0

Exp = mybir.ActivationFunctionType.Exp

last_exec_time_ns = None
_nc_cache = None


def _round_f32r(x):
    u = np.ascontiguousarray(x, dtype=np.float32).view(np.uint32)
    r = (u + np.uint32(0x7FF) + ((u >> np.uint32(12)) & np.uint32(1))) \
        & np.uint32(0xFFFFF000)
    return r.view(np.float32)


def _build_nc():
    nc = bacc.Bacc("TRN2", target_bir_lowering=False, debug=False)

    xT = nc.dram_tensor("xT", [B, D, S], F32R, kind="ExternalInput")
    wqkvT = nc.dram_tensor("wqkvT", [D, 6 * P], F32R, kind="ExternalInput")
    woT = nc.dram_tensor("woT", [CPC, D], F32R, kind="ExternalInput")
    cosT = nc.dram_tensor("cosT", [HD, S], F32, kind="ExternalInput")
    sinT = nc.dram_tensor("sinT", [HD, S], F32, kind="ExternalInput")
    rotL = nc.dram_tensor("rotL", [HD, HD], F32R, kind="ExternalInput")
    masks = nc.dram_tensor("masks", [4, P, SC], BF16, kind="ExternalInput")
    ones = nc.dram_tensor("ones", [P, P], F32R, kind="ExternalInput")
    yT = nc.dram_tensor("yT", [B, D, S], F32, kind="ExternalOutput")

    with tile.TileContext(nc) as tc:
        with tc.tile_pool(name="const", bufs=1) as constp, \
             tc.tile_pool(name="xp", bufs=17) as xp, \
             tc.tile_pool(name="qk", bufs=1) as qkp, \
             tc.tile_pool(name="vp", bufs=1) as vp, \
             tc.tile_pool(name="op", bufs=1) as op_, \
             tc.tile_pool(name="attn", bufs=3) as attnp, \
             tc.tile_pool(name="tmp", bufs=2) as tmpp, \
             tc.tile_pool(name="yt", bufs=2) as ytp, \
             tc.tile_pool(name="ps", bufs=4, space="PSUM") as psp, \
             tc.tile_pool(name="acc", bufs=4, space="PSUM") as accp:

            # ---- constants ----
            wq_sb = constp.tile([P, NDT, 6 * P], F32R)
            nc.sync.dma_start(wq_sb[:], wqkvT.rearrange("(o p) e -> p o e", p=P))
            wo_sb = constp.tile([P, CPC // P, D], F32R)
            nc.sync.dma_start(wo_sb[:], woT.rearrange("(o p) e -> p o e", p=P))
            cos_sb = constp.tile([P, S], F32)
            nc.sync.dma_start(cos_sb[:], cosT[:])
            sin_sb = constp.tile([P, S], F32)
            nc.sync.dma_start(sin_sb[:], sinT[:])
            rot_sb = constp.tile([P, P], F32R)
            nc.sync.dma_start(rot_sb[:], rotL[:])
            mask_sb = constp.tile([P, 4, SC], BF16)
            nc.sync.dma_start(mask_sb[:], masks.rearrange("m p q -> p m q"))
            ones_sb = constp.tile([P, P], F32R)
            nc.sync.dma_start(ones_sb[:], ones[:])

            for b in range(B):
                # ---- projections ----
                # qkT[e] for e in {q_h0, q_h1, k_h0, k_h1}: [128, S] transposed
                qkT = [qkp.tile([P, S], F32R, tag=f"qk{e}", name=f"qkT{e}")
                       for e in range(4)]
                # v natural [s_in=128, s_out=16, ch=256]
                v_sb = vp.tile([P, NDT, CPC], F32R, tag="v")
                for sc in range(NSC):
                    xts = []
                    for dt in range(NDT):
                        xt = xp.tile([P, SC], F32R, tag="xt")
                        nc.sync.dma_start(
                            xt[:], xT[b, dt * P:(dt + 1) * P, sc * SC:(sc + 1) * SC])
                        xts.append(xt)
                    for e in range(4):
                        pq = accp.tile([P, SC], F32, tag="acc")
                        for dt in range(NDT):
                            nc.tensor.matmul(pq[:],
                                             wq_sb[:, dt, e * P:(e + 1) * P],
                                             xts[dt][:],
                                             start=(dt == 0), stop=(dt == NDT - 1))
                        sl = slice(sc * SC, (sc + 1) * SC)
                        nc.scalar.copy(qkT[e][:, sl], pq[:])
                        # RoPE for this chunk, overlapped with projections
                        pr = psp.tile([P, SC], F32, tag="ps")
                        nc.tensor.matmul(pr[:], rot_sb[:], qkT[e][:, sl],
                                         start=True, stop=True)
                        tmp = tmpp.tile([P, SC], F32, tag="ropetmp")
                        nc.vector.tensor_tensor(tmp[:], pr[:], sin_sb[:, sl],
                                                AluOpType.mult)
                        nc.vector.tensor_tensor(qkT[e][:, sl], qkT[e][:, sl],
                                                cos_sb[:, sl], AluOpType.mult)
                        nc.vector.tensor_tensor(qkT[e][:, sl], qkT[e][:, sl],
                                                tmp[:], AluOpType.add)
                    for ss in range(SC // P):
                        pv = accp.tile([P, SC], F32, tag="acc")
                        pvv = pv[:, :CPC]
                        for dt in range(NDT):
                            nc.tensor.matmul(pvv,
                                             xts[dt][:, ss * P:(ss + 1) * P],
                                             wq_sb[:, dt, 4 * P:6 * P],
                                             start=(dt == 0), stop=(dt == NDT - 1))
                        nc.scalar.copy(v_sb[:, sc * (SC // P) + ss, :], pvv)

                # ---- attention per head ----
                oT = op_.tile([P, HPC, S], F32R, tag="o")
                for h in range(HPC):
                    qTh, kTh = qkT[h], qkT[2 + h]
                    for j in range(NSC):
                        jsl = slice(j * SC, (j + 1) * SC)
                        n_kv = (SC // P) * (j + 1)
                        po = accp.tile([P, SC], F32, tag="acc")
                        prs = accp.tile([P, SC], F32, tag="acc")
                        for t in range(n_kv):
                            pscore = psp.tile([P, SC], F32, tag="ps")
                            nc.tensor.matmul(pscore[:],
                                             kTh[:, t * P:(t + 1) * P],
                                             qTh[:, jsl],
                                             start=True, stop=True)
                            attn = attnp.tile([P, SC], F32R, tag="attn")
                            nc.scalar.activation(attn[:], pscore[:], Exp,
                                                 bias=0.0, scale=1.0)
                            dp = t - (SC // P) * j
                            if dp >= 0:  # staircase tile: apply causal mask
                                nc.vector.tensor_tensor(attn[:], attn[:],
                                                        mask_sb[:, dp, :],
                                                        AluOpType.mult)
                            nc.tensor.matmul(po[:],
                                             v_sb[:, t, h * HD:(h + 1) * HD],
                                             attn[:],
                                             start=(t == 0), stop=(t == n_kv - 1))
                            nc.tensor.matmul(prs[:], ones_sb[:], attn[:],
                                             start=(t == 0), stop=(t == n_kv - 1))
                        recip = tmpp.tile([P, SC], F32, tag="recip")
                        nc.vector.reciprocal(recip[:], prs[:])
                        nc.vector.tensor_tensor(oT[:, h, jsl], po[:], recip[:],
                                                AluOpType.mult)

                # ---- output projection: yT[e,s] = sum_ct woT.T @ oT ----
                for et in range(NDT):
                    for sc in range(NSC):
                        py = accp.tile([P, SC], F32, tag="acc")
                        for ct in range(HPC):
                            nc.tensor.matmul(py[:],
                                             wo_sb[:, ct, et * P:(et + 1) * P],
                                             oT[:, ct, sc * SC:(sc + 1) * SC],
                                             start=(ct == 0), stop=(ct == HPC - 1))
                        yt = ytp.tile([P, SC], F32, tag="yt")
                        nc.scalar.copy(yt[:], py[:])
                        nc.sync.dma_start(
                            yT[b, et * P:(et + 1) * P, sc * SC:(sc + 1) * SC],
                            yt[:])
    nc.finalize()
    return nc


def _host_inputs(x, wq, wk, wv, wo):
    """Build per-core input maps (host-side shard + transform)."""
    scale = 1.0 / np.sqrt(np.float32(HD))

    xTr = _round_f32r(np.ascontiguousarray(x.transpose(0, 2, 1)))

    # RoPE tables in [e, s] layout (same for every head)
    inv_freq = 1.0 / (ROPE_THETA ** (np.arange(0, HD, 2, dtype=np.float64) / HD))
    ang = np.arange(S, dtype=np.float64)[None, :] * inv_freq[:, None]  # [64, S]
    cosT = np.repeat(np.cos(ang), 2, axis=0).astype(np.float32)  # [128, S]
    sinT = np.repeat(np.sin(ang), 2, axis=0).astype(np.float32)

    # signed pair-swap: qrot[2i] = -q[2i+1], qrot[2i+1] = q[2i]
    # matmul computes qrot[m, s] = sum_k rotL[k, m] q[k, s]
    rotL = np.zeros((HD, HD), dtype=np.float32)
    for i in range(HD // 2):
        rotL[2 * i + 1, 2 * i] = -1.0
        rotL[2 * i, 2 * i + 1] = 1.0

    import ml_dtypes
    r = np.arange(P)[:, None]
    c = np.arange(SC)[None, :]
    masks = np.stack([(c >= p * P + r) for p in range(4)]) \
        .astype(ml_dtypes.bfloat16)  # [4, 128, 512]

    wq_s = _round_f32r(wq * scale)
    wk_s = _round_f32r(wk)
    wv_s = _round_f32r(wv)
    wo_s = _round_f32r(wo)

    in_maps = []
    for cix in range(NCORES):
        rows = slice(cix * CPC, (cix + 1) * CPC)  # head-channel rows
        blocks = []
        for h in range(HPC):
            hr = slice((cix * HPC + h) * HD, (cix * HPC + h + 1) * HD)
            blocks.append(wq_s[hr])   # q_h: [128, D]
        for h in range(HPC):
            hr = slice((cix * HPC + h) * HD, (cix * HPC + h + 1) * HD)
            blocks.append(wk_s[hr])
        blocks.append(wv_s[rows])     # v both heads: [256, D]
        wqkvT = np.ascontiguousarray(
            np.concatenate(blocks, axis=0).T)  # [D, 768]
        woT = np.ascontiguousarray(wo_s[:, rows].T)  # [256, D]
        in_maps.append({
            "xT": xTr,
            "wqkvT": wqkvT,
            "woT": woT,
            "cosT": cosT,
            "sinT": sinT,
            "rotL": rotL,
            "masks": masks,
            "ones": np.ones((P, P), dtype=np.float32),
        })
    return in_maps


def _get_nc():
    global _nc_cache
    if _nc_cache is None:
        _nc_cache = _build_nc()
    return _nc_cache


def kernel(x, wq, wk, wv, wo, _trace=False):
    global last_exec_time_ns
    nc = _get_nc()
    in_maps = _host_inputs(np.asarray(x, dtype=np.float32),
                           np.asarray(wq, dtype=np.float32),
                           np.asarray(wk, dtype=np.float32),
                           np.asarray(wv, dtype=np.float32),
                           np.asarray(wo, dtype=np.float32))
    res = run_bass_kernel_spmd(nc, in_maps, core_ids=list(range(NCORES)),
                               trace=_trace)
    last_exec_time_ns = res.exec_time_ns
    y = np.zeros((B, S, D), dtype=np.float64)
    for cix in range(NCORES):
        y += res.results[cix]["yT"].transpose(0, 2, 1).astype(np.float64)
    return y.astype(np.float32)


# revision 8
# speedup vs baseline: 1.2402x; 1.0311x over previous
"""Multi-head causal self-attention with RoPE on 8 Trainium2 NeuronCores.

Problem: x[2,2048,2048], wq/wk/wv/wo[2048,2048] fp32, 16 heads (hd=128),
interleaved RoPE, causal softmax, Megatron-style tensor parallelism over
heads: 2 heads per core, wo row-sharded, partial outputs summed on host.

All matmuls run as float32r (fp32 rounded to 11-bit mantissa; full PE rate
at moving-dim >= 256). Host pre-rounds DRAM inputs to f32r; on-device
producers write f32r directly.

Layout strategy (per core, per batch b):
  - host supplies xT = x^T [d, s] (f32r) and weight slices pre-transposed
  - projections: qT,kT per head via lhsT=w-tile [d,e], rhs=xT [d,s512]
    -> q^T,k^T [e=128, s] directly; v natural [s, e] via lhsT=xT-subtile
  - RoPE: qrotT = RotL.T @ qT (signed pair-swap as a matmul), then
    q_roped = qT*cosT + qrotT*sinT on DVE (tables indexed [e, s])
  - attention per (b, h), per q-block of 512:
      scoresT[kv=128, q=512] = kT-tile.T @ qT-block   (one matmul, d=128)
      attn = exp(scoresT) on ACT (-> f32r), staircase tiles masked (DVE mul)
      oT[d, q] += v-tile.T @ attn ; rowsumB[128, q] += ones.T @ attn
      oT_norm = oT * reciprocal(rowsumB)  (-> f32r)
  - output projection: yT[e, s] = sum_ct woT-tile.T @ oT  -> DRAM
  - host: y = sum over cores of yT^T
"""

import os
import sys

for _p in ("/opt/trn_rl_repo", "/root/.axon_site/_ro/trn_rl_repo"):
    if os.path.isdir(_p) and _p not in sys.path:
        sys.path.append(_p)

import numpy as np

import concourse.bacc as bacc
import concourse.mybir as mybir
import concourse.tile as tile
from concourse.alu_op_type import AluOpType
from concourse.bass_utils import run_bass_kernel_spmd

F32 = mybir.dt.float32
F32R = mybir.dt.float32r
BF16 = mybir.dt.bfloat16

B, S, D = 2, 2048, 2048
H, HD = 16, 128
NCORES = 8
HPC = H // NCORES            # heads per core = 2
CPC = HPC * HD               # channels per core = 256
P = 128
SC = 512                     # s-chunk for projections / q-block for attention
NSC = S // SC                # 4
NDT = D // P                 # 16 contraction tiles
ROPE_THETA = 10000.0

Exp = mybir.ActivationFunctionType.Exp

last_exec_time_ns = None
_nc_cache = None


def _round_f32r(x):
    u = np.ascontiguousarray(x, dtype=np.float32).view(np.uint32)
    r = (u + np.uint32(0x7FF) + ((u >> np.uint32(12)) & np.uint32(1))) \
        & np.uint32(0xFFFFF000)
    return r.view(np.float32)


def _build_nc():
    nc = bacc.Bacc("TRN2", target_bir_lowering=False, debug=False)

    xT = nc.dram_tensor("xT", [B, D, S], F32R, kind="ExternalInput")
    wqkvT = nc.dram_tensor("wqkvT", [D, 6 * P], F32R, kind="ExternalInput")
    woT = nc.dram_tensor("woT", [CPC, D], F32R, kind="ExternalInput")
    cosT = nc.dram_tensor("cosT", [HD, S], F32, kind="ExternalInput")
    sinT = nc.dram_tensor("sinT", [HD, S], F32, kind="ExternalInput")
    rotL = nc.dram_tensor("rotL", [HD, HD], F32R, kind="ExternalInput")
    masks = nc.dram_tensor("masks", [4, P, SC], BF16, kind="ExternalInput")
    ones = nc.dram_tensor("ones", [P, P], F32R, kind="ExternalInput")
    yT = nc.dram_tensor("yT", [B, D, S], F32, kind="ExternalOutput")

    with tile.TileContext(nc) as tc:
        with tc.tile_pool(name="const", bufs=1) as constp, \
             tc.tile_pool(name="xp", bufs=16) as xp, \
             tc.tile_pool(name="qk", bufs=1) as qkp, \
             tc.tile_pool(name="vp", bufs=1) as vp, \
             tc.tile_pool(name="op", bufs=1) as op_, \
             tc.tile_pool(name="attn", bufs=5) as attnp, \
             tc.tile_pool(name="tmp", bufs=2) as tmpp, \
             tc.tile_pool(name="yt", bufs=2) as ytp, \
             tc.tile_pool(name="ps", bufs=4, space="PSUM") as psp, \
             tc.tile_pool(name="acc", bufs=4, space="PSUM") as accp:

            # ---- constants (wq split per d-tile so matmuls start early;
            #      the rest deferred until after the first x-chunk DMAs) ----
            wq_sb = constp.tile([P, NDT, 6 * P], F32R)
            wqr = wqkvT.rearrange("(o p) e -> p o e", p=P)
            for dt in range(NDT):
                nc.sync.dma_start(wq_sb[:, dt, :], wqr[:, dt, :])
            wo_sb = constp.tile([P, CPC // P, D], F32R)
            cos_sb = constp.tile([P, S], F32)
            sin_sb = constp.tile([P, S], F32)
            rot_sb = constp.tile([P, P], F32R)
            mask_sb = constp.tile([P, 4, SC], BF16)
            ones_sb = constp.tile([P, P], F32R)

            def load_rest_of_consts():
                nc.sync.dma_start(rot_sb[:], rotL[:])
                nc.sync.dma_start(cos_sb[:], cosT[:])
                nc.sync.dma_start(sin_sb[:], sinT[:])
                nc.sync.dma_start(mask_sb[:], masks.rearrange("m p q -> p m q"))
                nc.sync.dma_start(ones_sb[:], ones[:])
                nc.sync.dma_start(wo_sb[:], woT.rearrange("(o p) e -> p o e", p=P))

            for b in range(B):
                # ---- projections ----
                # qkT[e] for e in {q_h0, q_h1, k_h0, k_h1}: [128, S] transposed
                qkT = [qkp.tile([P, S], F32R, tag=f"qk{e}", name=f"qkT{e}")
                       for e in range(4)]
                # v natural [s_in=128, s_out=16, ch=256]
                v_sb = vp.tile([P, NDT, CPC], F32R, tag="v")
                for sc in range(NSC):
                    xts = []
                    for dt in range(NDT):
                        xt = xp.tile([P, SC], F32R, tag="xt")
                        nc.sync.dma_start(
                            xt[:], xT[b, dt * P:(dt + 1) * P, sc * SC:(sc + 1) * SC])
                        xts.append(xt)
                    if b == 0 and sc == 0:
                        load_rest_of_consts()
                    for e in range(4):
                        pq = accp.tile([P, SC], F32, tag="acc")
                        for dt in range(NDT):
                            nc.tensor.matmul(pq[:],
                                             wq_sb[:, dt, e * P:(e + 1) * P],
                                             xts[dt][:],
                                             start=(dt == 0), stop=(dt == NDT - 1))
                        sl = slice(sc * SC, (sc + 1) * SC)
                        nc.scalar.copy(qkT[e][:, sl], pq[:])
                        # RoPE for this chunk, overlapped with projections
                        pr = psp.tile([P, SC], F32, tag="ps")
                        nc.tensor.matmul(pr[:], rot_sb[:], qkT[e][:, sl],
                                         start=True, stop=True)
                        tmp = tmpp.tile([P, SC], F32, tag="ropetmp")
                        nc.vector.tensor_tensor(tmp[:], pr[:], sin_sb[:, sl],
                                                AluOpType.mult)
                        nc.vector.tensor_tensor(qkT[e][:, sl], qkT[e][:, sl],
                                                cos_sb[:, sl], AluOpType.mult)
                        nc.vector.tensor_tensor(qkT[e][:, sl], qkT[e][:, sl],
                                                tmp[:], AluOpType.add)
                    for ss in range(SC // P):
                        pv = accp.tile([P, SC], F32, tag="acc")
                        pvv = pv[:, :CPC]
                        for dt in range(NDT):
                            nc.tensor.matmul(pvv,
                                             xts[dt][:, ss * P:(ss + 1) * P],
                                             wq_sb[:, dt, 4 * P:6 * P],
                                             start=(dt == 0), stop=(dt == NDT - 1))
                        nc.scalar.copy(v_sb[:, sc * (SC // P) + ss, :], pvv)

                # ---- attention: j outer, heads interleaved ----
                oT = op_.tile([P, HPC, S], F32R, tag="o")
                for j in range(NSC):
                    jsl = slice(j * SC, (j + 1) * SC)
                    n_kv = (SC // P) * (j + 1)
                    for h in range(HPC):
                        qTh, kTh = qkT[h], qkT[2 + h]
                        po = accp.tile([P, SC], F32, tag="acc")
                        prs = accp.tile([P, SC], F32, tag="acc")
                        for t in range(n_kv):
                            pscore = psp.tile([P, SC], F32, tag="ps")
                            nc.tensor.matmul(pscore[:],
                                             kTh[:, t * P:(t + 1) * P],
                                             qTh[:, jsl],
                                             start=True, stop=True)
                            attn = attnp.tile([P, SC], F32R, tag="attn")
                            nc.scalar.activation(attn[:], pscore[:], Exp,
                                                 bias=0.0, scale=1.0)
                            dp = t - (SC // P) * j
                            if dp >= 0:  # staircase tile: apply causal mask
                                nc.vector.tensor_tensor(attn[:], attn[:],
                                                        mask_sb[:, dp, :],
                                                        AluOpType.mult)
                            nc.tensor.matmul(po[:],
                                             v_sb[:, t, h * HD:(h + 1) * HD],
                                             attn[:],
                                             start=(t == 0), stop=(t == n_kv - 1))
                            nc.tensor.matmul(prs[:], ones_sb[:], attn[:],
                                             start=(t == 0), stop=(t == n_kv - 1))
                        recip = tmpp.tile([P, SC], F32, tag="recip")
                        nc.vector.reciprocal(recip[:], prs[:])
                        nc.vector.tensor_tensor(oT[:, h, jsl], po[:], recip[:],
                                                AluOpType.mult)

                # ---- output projection: yT[e,s] = sum_ct woT.T @ oT ----
                for et in range(NDT):
                    for sc in range(NSC):
                        py = accp.tile([P, SC], F32, tag="acc")
                        for ct in range(HPC):
                            nc.tensor.matmul(py[:],
                                             wo_sb[:, ct, et * P:(et + 1) * P],
                                             oT[:, ct, sc * SC:(sc + 1) * SC],
                                             start=(ct == 0), stop=(ct == HPC - 1))
                        yt = ytp.tile([P, SC], F32, tag="yt")
                        nc.scalar.copy(yt[:], py[:])
                        nc.sync.dma_start(
                            yT[b, et * P:(et + 1) * P, sc * SC:(sc + 1) * SC],
                            yt[:])
    nc.finalize()
    return nc


def _host_inputs(x, wq, wk, wv, wo):
    """Build per-core input maps (host-side shard + transform)."""
    scale = 1.0 / np.sqrt(np.float32(HD))

    xTr = _round_f32r(np.ascontiguousarray(x.transpose(0, 2, 1)))

    # RoPE tables in [e, s] layout (same for every head)
    inv_freq = 1.0 / (ROPE_THETA ** (np.arange(0, HD, 2, dtype=np.float64) / HD))
    ang = np.arange(S, dtype=np.float64)[None, :] * inv_freq[:, None]  # [64, S]
    cosT = np.repeat(np.cos(ang), 2, axis=0).astype(np.float32)  # [128, S]
    sinT = np.repeat(np.sin(ang), 2, axis=0).astype(np.float32)

    # signed pair-swap: qrot[2i] = -q[2i+1], qrot[2i+1] = q[2i]
    # matmul computes qrot[m, s] = sum_k rotL[k, m] q[k, s]
    rotL = np.zeros((HD, HD), dtype=np.float32)
    for i in range(HD // 2):
        rotL[2 * i + 1, 2 * i] = -1.0
        rotL[2 * i, 2 * i + 1] = 1.0

    import ml_dtypes
    r = np.arange(P)[:, None]
    c = np.arange(SC)[None, :]
    masks = np.stack([(c >= p * P + r) for p in range(4)]) \
        .astype(ml_dtypes.bfloat16)  # [4, 128, 512]

    wq_s = _round_f32r(wq * scale)
    wk_s = _round_f32r(wk)
    wv_s = _round_f32r(wv)
    wo_s = _round_f32r(wo)

    in_maps = []
    for cix in range(NCORES):
        rows = slice(cix * CPC, (cix + 1) * CPC)  # head-channel rows
        blocks = []
        for h in range(HPC):
            hr = slice((cix * HPC + h) * HD, (cix * HPC + h + 1) * HD)
            blocks.append(wq_s[hr])   # q_h: [128, D]
        for h in range(HPC):
            hr = slice((cix * HPC + h) * HD, (cix * HPC + h + 1) * HD)
            blocks.append(wk_s[hr])
        blocks.append(wv_s[rows])     # v both heads: [256, D]
        wqkvT = np.ascontiguousarray(
            np.concatenate(blocks, axis=0).T)  # [D, 768]
        woT = np.ascontiguousarray(wo_s[:, rows].T)  # [256, D]
        in_maps.append({
            "xT": xTr,
            "wqkvT": wqkvT,
            "woT": woT,
            "cosT": cosT,
            "sinT": sinT,
            "rotL": rotL,
            "masks": masks,
            "ones": np.ones((P, P), dtype=np.float32),
        })
    return in_maps


def _get_nc():
    global _nc_cache
    if _nc_cache is None:
        _nc_cache = _build_nc()
    return _nc_cache


def kernel(x, wq, wk, wv, wo, _trace=False):
    global last_exec_time_ns
    nc = _get_nc()
    in_maps = _host_inputs(np.asarray(x, dtype=np.float32),
                           np.asarray(wq, dtype=np.float32),
                           np.asarray(wk, dtype=np.float32),
                           np.asarray(wv, dtype=np.float32),
                           np.asarray(wo, dtype=np.float32))
    res = run_bass_kernel_spmd(nc, in_maps, core_ids=list(range(NCORES)),
                               trace=_trace)
    last_exec_time_ns = res.exec_time_ns
    y = np.zeros((B, S, D), dtype=np.float64)
    for cix in range(NCORES):
        y += res.results[cix]["yT"].transpose(0, 2, 1).astype(np.float64)
    return y.astype(np.float32)


# revision 11
# speedup vs baseline: 1.5792x; 1.2733x over previous
"""Multi-head causal self-attention with RoPE on 8 Trainium2 NeuronCores.

Problem: x[2,2048,2048], wq/wk/wv/wo[2048,2048] fp32, 16 heads (hd=128),
interleaved RoPE, causal softmax, Megatron-style tensor parallelism over
heads: 2 heads per core, wo row-sharded, partial outputs summed on host.

All matmuls run as float32r (fp32 rounded to 11-bit mantissa; ~1 cycle/row
warm when back-to-back). Host pre-rounds DRAM inputs to f32r; on-device
producers write f32r directly.

Layout strategy (per core, per batch b):
  - host supplies xT = x^T [d, s] (f32r) and weight slices pre-transposed
  - projections: qT,kT per head via lhsT=w-tile [d,e], rhs=xT [d,s512]
    -> q^T,k^T [e=128, s] directly; v natural [s, e] via lhsT=xT-subtile;
    RoPE fused right after each projection chunk:
    qrotT = RotL.T @ qT (signed pair-swap as a matmul), then
    q_roped = qT*cosT + qrotT*sinT on DVE (tables indexed [e, s])
  - attention per (b, j-block of 512 q), heads interleaved:
      scoresT[kv=128, q=512] = kT-tile.T @ qT-block   (one matmul, d=128)
      staircase tiles compute only valid columns [delta:512]
      attn = exp(scoresT) on ACT (-> f32r); triangle mask on 128-col band
      oT[d, q] += v-tile.T @ attn ; rowsumB[128, q] += ones.T @ attn
      oT_norm = oT * reciprocal_approx_fast(rowsumB)  (-> f32r)
  - output projection: yT[e, s] = sum_ct woT-tile.T @ oT  -> DRAM
  - host: y = sum over cores of yT^T
"""

import os
import sys

for _p in ("/opt/trn_rl_repo", "/root/.axon_site/_ro/trn_rl_repo"):
    if os.path.isdir(_p) and _p not in sys.path:
        sys.path.append(_p)

import numpy as np

import concourse.bacc as bacc
import concourse.mybir as mybir
import concourse.tile as tile
from concourse.alu_op_type import AluOpType
from concourse.bass_utils import run_bass_kernel_spmd

F32 = mybir.dt.float32
F32R = mybir.dt.float32r
BF16 = mybir.dt.bfloat16

B, S, D = 2, 2048, 2048
H, HD = 16, 128
NCORES = 8
HPC = H // NCORES            # heads per core = 2
CPC = HPC * HD               # channels per core = 256
P = 128
SC = 512                     # s-chunk for projections / q-block for attention
NSC = S // SC                # 4
NDT = D // P                 # 16 contraction tiles
NG = 2                       # x-tile DMA group: d-tiles per DMA
ROPE_THETA = 10000.0

Exp = mybir.ActivationFunctionType.Exp

last_exec_time_ns = None
_nc_cache = None


def _round_f32r(x):
    u = np.ascontiguousarray(x, dtype=np.float32).view(np.uint32)
    r = (u + np.uint32(0x7FF) + ((u >> np.uint32(12)) & np.uint32(1))) \
        & np.uint32(0xFFFFF000)
    return r.view(np.float32)


def _build_nc():
    nc = bacc.Bacc("TRN2", target_bir_lowering=False, debug=False)

    xT = nc.dram_tensor("xT", [B, D, S], F32R, kind="ExternalInput")
    wqkvT = nc.dram_tensor("wqkvT", [D, 6 * P], F32R, kind="ExternalInput")
    woT = nc.dram_tensor("woT", [CPC, D], F32R, kind="ExternalInput")
    cosT = nc.dram_tensor("cosT", [HD, S], F32, kind="ExternalInput")
    sinT = nc.dram_tensor("sinT", [HD, S], F32, kind="ExternalInput")
    rotL = nc.dram_tensor("rotL", [HD, HD], F32R, kind="ExternalInput")
    trimask = nc.dram_tensor("trimask", [P, P], BF16, kind="ExternalInput")
    ones = nc.dram_tensor("ones", [P, P], F32R, kind="ExternalInput")
    yT = nc.dram_tensor("yT", [B, D, S], F32, kind="ExternalOutput")

    xTr = xT.rearrange("b (o p) s -> b p o s", p=P)

    with tile.TileContext(nc) as tc:
        with tc.tile_pool(name="const", bufs=1) as constp, \
             tc.tile_pool(name="xp", bufs=9) as xp, \
             tc.tile_pool(name="qk", bufs=1) as qkp, \
             tc.tile_pool(name="vp", bufs=1) as vp, \
             tc.tile_pool(name="op", bufs=1) as op_, \
             tc.tile_pool(name="attn", bufs=4) as attnp, \
             tc.tile_pool(name="tmp", bufs=2) as tmpp, \
             tc.tile_pool(name="yt", bufs=2) as ytp, \
             tc.tile_pool(name="ps", bufs=4, space="PSUM") as psp, \
             tc.tile_pool(name="acc", bufs=4, space="PSUM") as accp:

            # ---- constants (wq split per d-tile so matmuls start early;
            #      the rest deferred until after the first x-chunk DMAs) ----
            wq_sb = constp.tile([P, NDT, 6 * P], F32R)
            wqr = wqkvT.rearrange("(o p) e -> p o e", p=P)
            for dt in range(NDT):
                nc.sync.dma_start(wq_sb[:, dt, :], wqr[:, dt, :])
            wo_sb = constp.tile([P, CPC // P, D], F32R)
            cos_sb = constp.tile([P, S], F32)
            sin_sb = constp.tile([P, S], F32)
            rot_sb = constp.tile([P, P], F32R)
            mask_sb = constp.tile([P, P], BF16)
            ones_sb = constp.tile([P, P], F32R)

            def load_rest_of_consts():
                nc.sync.dma_start(rot_sb[:], rotL[:])
                nc.sync.dma_start(cos_sb[:], cosT[:])
                nc.sync.dma_start(sin_sb[:], sinT[:])
                nc.sync.dma_start(mask_sb[:], trimask[:])
                nc.sync.dma_start(ones_sb[:], ones[:])
                nc.sync.dma_start(wo_sb[:], woT.rearrange("(o p) e -> p o e", p=P))

            for b in range(B):
                # ---- projections (+ fused RoPE) ----
                # qkT[e] for e in {q_h0, q_h1, k_h0, k_h1}: [128, S] transposed
                qkT = [qkp.tile([P, S], F32R, tag=f"qk{e}", name=f"qkT{e}")
                       for e in range(4)]
                # v natural [s_in=128, s_out=16, ch=256]
                v_sb = vp.tile([P, NDT, CPC], F32R, tag="v")
                for sc in range(NSC):
                    xts = []
                    for g in range(NDT // NG):
                        xt = xp.tile([P, NG, SC], F32R, tag="xt")
                        nc.sync.dma_start(
                            xt[:], xTr[b, :, g * NG:(g + 1) * NG,
                                       sc * SC:(sc + 1) * SC])
                        xts.append(xt)
                    if b == 0 and sc == 0:
                        load_rest_of_consts()
                    for e in range(4):
                        pq = accp.tile([P, SC], F32, tag="acc")
                        for dt in range(NDT):
                            nc.tensor.matmul(pq[:],
                                             wq_sb[:, dt, e * P:(e + 1) * P],
                                             xts[dt // NG][:, dt % NG, :],
                                             start=(dt == 0), stop=(dt == NDT - 1))
                        sl = slice(sc * SC, (sc + 1) * SC)
                        nc.scalar.copy(qkT[e][:, sl], pq[:])
                        # RoPE for this chunk, overlapped with projections
                        pr = psp.tile([P, SC], F32, tag="ps")
                        nc.tensor.matmul(pr[:], rot_sb[:], qkT[e][:, sl],
                                         start=True, stop=True)
                        tmp = tmpp.tile([P, SC], F32, tag="ropetmp")
                        nc.vector.tensor_tensor(tmp[:], pr[:], sin_sb[:, sl],
                                                AluOpType.mult)
                        nc.vector.tensor_tensor(qkT[e][:, sl], qkT[e][:, sl],
                                                cos_sb[:, sl], AluOpType.mult)
                        nc.vector.tensor_tensor(qkT[e][:, sl], qkT[e][:, sl],
                                                tmp[:], AluOpType.add)
                    for ss in range(SC // P):
                        pv = accp.tile([P, SC], F32, tag="acc")
                        pvv = pv[:, :CPC]
                        for dt in range(NDT):
                            nc.tensor.matmul(pvv,
                                             xts[dt // NG][:, dt % NG,
                                                           ss * P:(ss + 1) * P],
                                             wq_sb[:, dt, 4 * P:6 * P],
                                             start=(dt == 0), stop=(dt == NDT - 1))
                        nc.scalar.copy(v_sb[:, sc * (SC // P) + ss, :], pvv)

                # ---- attention: j outer, heads interleaved ----
                oT = op_.tile([P, HPC, S], F32R, tag="o")
                for j in range(NSC):
                    jsl = slice(j * SC, (j + 1) * SC)
                    n_kv = (SC // P) * (j + 1)
                    for h in range(HPC):
                        qTh, kTh = qkT[h], qkT[2 + h]
                        po = accp.tile([P, SC], F32, tag="acc")
                        prs = accp.tile([P, SC], F32, tag="acc")
                        for t in range(n_kv):
                            dp = t - (SC // P) * j
                            dlt = max(dp, 0) * P  # first valid column
                            vsl = slice(j * SC + dlt, (j + 1) * SC)
                            pscore = psp.tile([P, SC], F32, tag="ps")
                            nc.tensor.matmul(pscore[:, dlt:],
                                             kTh[:, t * P:(t + 1) * P],
                                             qTh[:, vsl],
                                             start=True, stop=True)
                            attn = attnp.tile([P, SC], F32R, tag="attn")
                            nc.scalar.activation(attn[:, dlt:], pscore[:, dlt:],
                                                 Exp, bias=0.0, scale=1.0)
                            if dp >= 0:  # triangle mask on the 128-col band
                                nc.vector.tensor_tensor(
                                    attn[:, dlt:dlt + P], attn[:, dlt:dlt + P],
                                    mask_sb[:], AluOpType.mult)
                            nc.tensor.matmul(po[:, dlt:],
                                             v_sb[:, t, h * HD:(h + 1) * HD],
                                             attn[:, dlt:],
                                             start=(t == 0), stop=(t == n_kv - 1),
                                             skip_group_check=True)
                            nc.tensor.matmul(prs[:, dlt:], ones_sb[:],
                                             attn[:, dlt:],
                                             start=(t == 0), stop=(t == n_kv - 1),
                                             skip_group_check=True)
                        recip = tmpp.tile([P, SC], F32, tag="recip")
                        nc.vector.reciprocal_approx_fast(recip[:], prs[:])
                        nc.vector.tensor_tensor(oT[:, h, jsl], po[:], recip[:],
                                                AluOpType.mult)

                # ---- output projection: yT[e,s] = sum_ct woT.T @ oT ----
                for et in range(NDT):
                    for half in range(2):
                        yt = ytp.tile([P, 2, SC], F32, tag="yt")
                        for si in range(2):
                            sc = half * 2 + si
                            py = accp.tile([P, SC], F32, tag="acc")
                            for ct in range(HPC):
                                nc.tensor.matmul(
                                    py[:],
                                    wo_sb[:, ct, et * P:(et + 1) * P],
                                    oT[:, ct, sc * SC:(sc + 1) * SC],
                                    start=(ct == 0), stop=(ct == HPC - 1))
                            nc.scalar.copy(yt[:, si, :], py[:])
                        nc.sync.dma_start(
                            yT[b, et * P:(et + 1) * P,
                               half * 2 * SC:(half * 2 + 2) * SC].rearrange(
                                "p (n q) -> p n q", n=2),
                            yt[:])
    nc.finalize()
    return nc


def _host_inputs(x, wq, wk, wv, wo):
    """Build per-core input maps (host-side shard + transform)."""
    scale = 1.0 / np.sqrt(np.float32(HD))

    xTr = _round_f32r(np.ascontiguousarray(x.transpose(0, 2, 1)))

    # RoPE tables in [e, s] layout (same for every head)
    inv_freq = 1.0 / (ROPE_THETA ** (np.arange(0, HD, 2, dtype=np.float64) / HD))
    ang = np.arange(S, dtype=np.float64)[None, :] * inv_freq[:, None]  # [64, S]
    cosT = np.repeat(np.cos(ang), 2, axis=0).astype(np.float32)  # [128, S]
    sinT = np.repeat(np.sin(ang), 2, axis=0).astype(np.float32)

    # signed pair-swap: qrot[2i] = -q[2i+1], qrot[2i+1] = q[2i]
    # matmul computes qrot[m, s] = sum_k rotL[k, m] q[k, s]
    rotL = np.zeros((HD, HD), dtype=np.float32)
    for i in range(HD // 2):
        rotL[2 * i + 1, 2 * i] = -1.0
        rotL[2 * i, 2 * i + 1] = 1.0

    import ml_dtypes
    r = np.arange(P)[:, None]
    c = np.arange(P)[None, :]
    trimask = (c >= r).astype(ml_dtypes.bfloat16)  # [128,128] upper-right valid

    wq_s = _round_f32r(wq * scale)
    wk_s = _round_f32r(wk)
    wv_s = _round_f32r(wv)
    wo_s = _round_f32r(wo)

    in_maps = []
    for cix in range(NCORES):
        rows = slice(cix * CPC, (cix + 1) * CPC)  # head-channel rows
        blocks = []
        for h in range(HPC):
            hr = slice((cix * HPC + h) * HD, (cix * HPC + h + 1) * HD)
            blocks.append(wq_s[hr])   # q_h: [128, D]
        for h in range(HPC):
            hr = slice((cix * HPC + h) * HD, (cix * HPC + h + 1) * HD)
            blocks.append(wk_s[hr])
        blocks.append(wv_s[rows])     # v both heads: [256, D]
        wqkvT = np.ascontiguousarray(
            np.concatenate(blocks, axis=0).T)  # [D, 768]
        woT = np.ascontiguousarray(wo_s[:, rows].T)  # [256, D]
        in_maps.append({
            "xT": xTr,
            "wqkvT": wqkvT,
            "woT": woT,
            "cosT": cosT,
            "sinT": sinT,
            "rotL": rotL,
            "trimask": trimask,
            "ones": np.ones((P, P), dtype=np.float32),
        })
    return in_maps


def _get_nc():
    global _nc_cache
    if _nc_cache is None:
        _nc_cache = _build_nc()
    return _nc_cache


def kernel(x, wq, wk, wv, wo, _trace=False):
    global last_exec_time_ns
    nc = _get_nc()
    in_maps = _host_inputs(np.asarray(x, dtype=np.float32),
                           np.asarray(wq, dtype=np.float32),
                           np.asarray(wk, dtype=np.float32),
                           np.asarray(wv, dtype=np.float32),
                           np.asarray(wo, dtype=np.float32))
    res = run_bass_kernel_spmd(nc, in_maps, core_ids=list(range(NCORES)),
                               trace=_trace)
    last_exec_time_ns = res.exec_time_ns
    y = np.zeros((B, S, D), dtype=np.float64)
    for cix in range(NCORES):
        y += res.results[cix]["yT"].transpose(0, 2, 1).astype(np.float64)
    return y.astype(np.float32)


# revision 12
# speedup vs baseline: 1.6243x; 1.0286x over previous
"""Multi-head causal self-attention with RoPE on 8 Trainium2 NeuronCores.

Problem: x[2,2048,2048], wq/wk/wv/wo[2048,2048] fp32, 16 heads (hd=128),
interleaved RoPE, causal softmax, Megatron-style tensor parallelism over
heads: 2 heads per core, wo row-sharded, partial outputs summed on host.

All matmuls run as float32r (fp32 rounded to 11-bit mantissa; ~1 cycle/row
warm when back-to-back). Host pre-rounds DRAM inputs to f32r; on-device
producers write f32r directly.

Layout strategy (per core, per batch b):
  - host supplies xT = x^T [d, s] (f32r) and weight slices pre-transposed
  - projections: qT,kT per head via lhsT=w-tile [d,e], rhs=xT [d,s512]
    -> q^T,k^T [e=128, s] directly; v natural [s, e] via lhsT=xT-subtile;
    RoPE fused right after each projection chunk:
    qrotT = RotL.T @ qT (signed pair-swap as a matmul), then
    q_roped = qT*cosT + qrotT*sinT on DVE (tables indexed [e, s])
  - attention per (b, j-block of 512 q), heads interleaved:
      scoresT[kv=128, q=512] = kT-tile.T @ qT-block   (one matmul, d=128)
      staircase tiles compute only valid columns [delta:512]
      attn = exp(scoresT) on ACT (-> f32r); triangle mask on 128-col band
      oT[d, q] += v-tile.T @ attn ; rowsumB[128, q] += ones.T @ attn
      oT_norm = oT * reciprocal_approx_fast(rowsumB)  (-> f32r)
  - output projection: yT[e, s] = sum_ct woT-tile.T @ oT  -> DRAM
  - host: y = sum over cores of yT^T
"""

import os
import sys

for _p in ("/opt/trn_rl_repo", "/root/.axon_site/_ro/trn_rl_repo"):
    if os.path.isdir(_p) and _p not in sys.path:
        sys.path.append(_p)

import numpy as np

import concourse.bacc as bacc
import concourse.mybir as mybir
import concourse.tile as tile
from concourse.alu_op_type import AluOpType
from concourse.bass_utils import run_bass_kernel_spmd

F32 = mybir.dt.float32
F32R = mybir.dt.float32r
BF16 = mybir.dt.bfloat16

B, S, D = 2, 2048, 2048
H, HD = 16, 128
NCORES = 8
HPC = H // NCORES            # heads per core = 2
CPC = HPC * HD               # channels per core = 256
P = 128
SC = 512                     # s-chunk for projections / q-block for attention
NSC = S // SC                # 4
NDT = D // P                 # 16 contraction tiles
NG = 2                       # x-tile DMA group: d-tiles per DMA
ROPE_THETA = 10000.0

Exp = mybir.ActivationFunctionType.Exp

last_exec_time_ns = None
_nc_cache = None


def _round_f32r(x):
    u = np.ascontiguousarray(x, dtype=np.float32).view(np.uint32)
    r = (u + np.uint32(0x7FF) + ((u >> np.uint32(12)) & np.uint32(1))) \
        & np.uint32(0xFFFFF000)
    return r.view(np.float32)


def _build_nc():
    nc = bacc.Bacc("TRN2", target_bir_lowering=False, debug=False)

    xT = nc.dram_tensor("xT", [B, D, S], F32R, kind="ExternalInput")
    wqkvT = nc.dram_tensor("wqkvT", [D, 6 * P], F32R, kind="ExternalInput")
    woT = nc.dram_tensor("woT", [CPC, D], F32R, kind="ExternalInput")
    cosT = nc.dram_tensor("cosT", [HD, S], F32, kind="ExternalInput")
    sinT = nc.dram_tensor("sinT", [HD, S], F32, kind="ExternalInput")
    rotL = nc.dram_tensor("rotL", [HD, HD], F32R, kind="ExternalInput")
    trimask = nc.dram_tensor("trimask", [P, P], BF16, kind="ExternalInput")
    ones = nc.dram_tensor("ones", [P, P], F32R, kind="ExternalInput")
    yT = nc.dram_tensor("yT", [B, D, S], F32, kind="ExternalOutput")

    xTr = xT.rearrange("b (o p) s -> b p o s", p=P)

    with tile.TileContext(nc) as tc:
        with tc.tile_pool(name="const", bufs=1) as constp, \
             tc.tile_pool(name="xp", bufs=9) as xp, \
             tc.tile_pool(name="qk", bufs=1) as qkp, \
             tc.tile_pool(name="vp", bufs=1) as vp, \
             tc.tile_pool(name="op", bufs=1) as op_, \
             tc.tile_pool(name="attn", bufs=4) as attnp, \
             tc.tile_pool(name="tmp", bufs=2) as tmpp, \
             tc.tile_pool(name="yt", bufs=2) as ytp, \
             tc.tile_pool(name="ps", bufs=4, space="PSUM") as psp, \
             tc.tile_pool(name="acc", bufs=4, space="PSUM") as accp:

            # ---- constants (wq split per d-tile so matmuls start early;
            #      the rest deferred until after the first x-chunk DMAs) ----
            wq_sb = constp.tile([P, NDT, 6 * P], F32R)
            wqr = wqkvT.rearrange("(o p) e -> p o e", p=P)
            for dt in range(NDT):
                nc.sync.dma_start(wq_sb[:, dt, :], wqr[:, dt, :])
            wo_sb = constp.tile([P, CPC // P, D], F32R)
            cos_sb = constp.tile([P, S], F32)
            sin_sb = constp.tile([P, S], F32)
            rot_sb = constp.tile([P, P], F32R)
            mask_sb = constp.tile([P, P], BF16)
            ones_sb = constp.tile([P, P], F32R)

            def load_rest_of_consts():
                nc.sync.dma_start(rot_sb[:], rotL[:])
                nc.sync.dma_start(cos_sb[:], cosT[:])
                nc.sync.dma_start(sin_sb[:], sinT[:])
                nc.sync.dma_start(mask_sb[:], trimask[:])
                nc.sync.dma_start(ones_sb[:], ones[:])
                nc.sync.dma_start(wo_sb[:], woT.rearrange("(o p) e -> p o e", p=P))

            for b in range(B):
                # ---- projections (+ fused RoPE) ----
                # qkT[e] for e in {q_h0, q_h1, k_h0, k_h1}: [128, S] transposed
                qkT = [qkp.tile([P, S], F32R, tag=f"qk{e}", name=f"qkT{e}")
                       for e in range(4)]
                # v natural [s_in=128, s_out=16, ch=256]
                v_sb = vp.tile([P, NDT, CPC], F32R, tag="v")
                for sc in range(NSC):
                    xts = []
                    for g in range(NDT // NG):
                        xt = xp.tile([P, NG, SC], F32R, tag="xt")
                        nc.gpsimd.dma_start(
                            xt[:], xTr[b, :, g * NG:(g + 1) * NG,
                                       sc * SC:(sc + 1) * SC])
                        xts.append(xt)
                    if b == 0 and sc == 0:
                        load_rest_of_consts()
                    for e in range(4):
                        pq = accp.tile([P, SC], F32, tag="acc")
                        for dt in range(NDT):
                            nc.tensor.matmul(pq[:],
                                             wq_sb[:, dt, e * P:(e + 1) * P],
                                             xts[dt // NG][:, dt % NG, :],
                                             start=(dt == 0), stop=(dt == NDT - 1))
                        sl = slice(sc * SC, (sc + 1) * SC)
                        nc.scalar.copy(qkT[e][:, sl], pq[:])
                        # RoPE for this chunk, overlapped with projections
                        pr = psp.tile([P, SC], F32, tag="ps")
                        nc.tensor.matmul(pr[:], rot_sb[:], qkT[e][:, sl],
                                         start=True, stop=True)
                        tmp = tmpp.tile([P, SC], F32, tag="ropetmp")
                        nc.vector.tensor_tensor(tmp[:], pr[:], sin_sb[:, sl],
                                                AluOpType.mult)
                        nc.vector.tensor_tensor(qkT[e][:, sl], qkT[e][:, sl],
                                                cos_sb[:, sl], AluOpType.mult)
                        nc.vector.tensor_tensor(qkT[e][:, sl], qkT[e][:, sl],
                                                tmp[:], AluOpType.add)
                    for ss in range(SC // P):
                        pv = accp.tile([P, SC], F32, tag="acc")
                        pvv = pv[:, :CPC]
                        for dt in range(NDT):
                            nc.tensor.matmul(pvv,
                                             xts[dt // NG][:, dt % NG,
                                                           ss * P:(ss + 1) * P],
                                             wq_sb[:, dt, 4 * P:6 * P],
                                             start=(dt == 0), stop=(dt == NDT - 1))
                        nc.scalar.copy(v_sb[:, sc * (SC // P) + ss, :], pvv)

                # ---- attention: j outer, heads interleaved ----
                oT = op_.tile([P, HPC, S], F32R, tag="o")
                for j in range(NSC):
                    jsl = slice(j * SC, (j + 1) * SC)
                    n_kv = (SC // P) * (j + 1)
                    for h in range(HPC):
                        qTh, kTh = qkT[h], qkT[2 + h]
                        po = accp.tile([P, SC], F32, tag="acc")
                        prs = accp.tile([P, SC], F32, tag="acc")
                        for t in range(n_kv):
                            dp = t - (SC // P) * j
                            dlt = max(dp, 0) * P  # first valid column
                            vsl = slice(j * SC + dlt, (j + 1) * SC)
                            pscore = psp.tile([P, SC], F32, tag="ps")
                            nc.tensor.matmul(pscore[:, dlt:],
                                             kTh[:, t * P:(t + 1) * P],
                                             qTh[:, vsl],
                                             start=True, stop=True)
                            attn = attnp.tile([P, SC], F32R, tag="attn")
                            nc.scalar.activation(attn[:, dlt:], pscore[:, dlt:],
                                                 Exp, bias=0.0, scale=1.0)
                            if dp >= 0:  # triangle mask on the 128-col band
                                nc.vector.tensor_tensor(
                                    attn[:, dlt:dlt + P], attn[:, dlt:dlt + P],
                                    mask_sb[:], AluOpType.mult)
                            nc.tensor.matmul(po[:, dlt:],
                                             v_sb[:, t, h * HD:(h + 1) * HD],
                                             attn[:, dlt:],
                                             start=(t == 0), stop=(t == n_kv - 1),
                                             skip_group_check=True)
                            nc.tensor.matmul(prs[:, dlt:], ones_sb[:],
                                             attn[:, dlt:],
                                             start=(t == 0), stop=(t == n_kv - 1),
                                             skip_group_check=True)
                        recip = tmpp.tile([P, SC], F32, tag="recip")
                        nc.vector.reciprocal_approx_fast(recip[:], prs[:])
                        nc.vector.tensor_tensor(oT[:, h, jsl], po[:], recip[:],
                                                AluOpType.mult)

                    # ---- output projection for this q-block:
                    #      yT[e, jsl] = sum_ct woT.T @ oT ----
                    for eh in range(NDT // 2):
                        yt = ytp.tile([P, 2, SC], F32, tag="yt")
                        for si in range(2):
                            et = eh * 2 + si
                            py = accp.tile([P, SC], F32, tag="acc")
                            for ct in range(HPC):
                                nc.tensor.matmul(
                                    py[:],
                                    wo_sb[:, ct, et * P:(et + 1) * P],
                                    oT[:, ct, jsl],
                                    start=(ct == 0), stop=(ct == HPC - 1))
                            nc.scalar.copy(yt[:, si, :], py[:])
                        nc.sync.dma_start(
                            yT[b, eh * 2 * P:(eh + 1) * 2 * P, jsl]
                            .rearrange("(n p) q -> p n q", p=P),
                            yt[:])
    nc.finalize()
    return nc


def _host_inputs(x, wq, wk, wv, wo):
    """Build per-core input maps (host-side shard + transform)."""
    scale = 1.0 / np.sqrt(np.float32(HD))

    xTr = _round_f32r(np.ascontiguousarray(x.transpose(0, 2, 1)))

    # RoPE tables in [e, s] layout (same for every head)
    inv_freq = 1.0 / (ROPE_THETA ** (np.arange(0, HD, 2, dtype=np.float64) / HD))
    ang = np.arange(S, dtype=np.float64)[None, :] * inv_freq[:, None]  # [64, S]
    cosT = np.repeat(np.cos(ang), 2, axis=0).astype(np.float32)  # [128, S]
    sinT = np.repeat(np.sin(ang), 2, axis=0).astype(np.float32)

    # signed pair-swap: qrot[2i] = -q[2i+1], qrot[2i+1] = q[2i]
    # matmul computes qrot[m, s] = sum_k rotL[k, m] q[k, s]
    rotL = np.zeros((HD, HD), dtype=np.float32)
    for i in range(HD // 2):
        rotL[2 * i + 1, 2 * i] = -1.0
        rotL[2 * i, 2 * i + 1] = 1.0

    import ml_dtypes
    r = np.arange(P)[:, None]
    c = np.arange(P)[None, :]
    trimask = (c >= r).astype(ml_dtypes.bfloat16)  # [128,128] upper-right valid

    wq_s = _round_f32r(wq * scale)
    wk_s = _round_f32r(wk)
    wv_s = _round_f32r(wv)
    wo_s = _round_f32r(wo)

    in_maps = []
    for cix in range(NCORES):
        rows = slice(cix * CPC, (cix + 1) * CPC)  # head-channel rows
        blocks = []
        for h in range(HPC):
            hr = slice((cix * HPC + h) * HD, (cix * HPC + h + 1) * HD)
            blocks.append(wq_s[hr])   # q_h: [128, D]
        for h in range(HPC):
            hr = slice((cix * HPC + h) * HD, (cix * HPC + h + 1) * HD)
            blocks.append(wk_s[hr])
        blocks.append(wv_s[rows])     # v both heads: [256, D]
        wqkvT = np.ascontiguousarray(
            np.concatenate(blocks, axis=0).T)  # [D, 768]
        woT = np.ascontiguousarray(wo_s[:, rows].T)  # [256, D]
        in_maps.append({
            "xT": xTr,
            "wqkvT": wqkvT,
            "woT": woT,
            "cosT": cosT,
            "sinT": sinT,
            "rotL": rotL,
            "trimask": trimask,
            "ones": np.ones((P, P), dtype=np.float32),
        })
    return in_maps


def _get_nc():
    global _nc_cache
    if _nc_cache is None:
        _nc_cache = _build_nc()
    return _nc_cache


def kernel(x, wq, wk, wv, wo, _trace=False):
    global last_exec_time_ns
    nc = _get_nc()
    in_maps = _host_inputs(np.asarray(x, dtype=np.float32),
                           np.asarray(wq, dtype=np.float32),
                           np.asarray(wk, dtype=np.float32),
                           np.asarray(wv, dtype=np.float32),
                           np.asarray(wo, dtype=np.float32))
    res = run_bass_kernel_spmd(nc, in_maps, core_ids=list(range(NCORES)),
                               trace=_trace)
    last_exec_time_ns = res.exec_time_ns
    y = np.zeros((B, S, D), dtype=np.float64)
    for cix in range(NCORES):
        y += res.results[cix]["yT"].transpose(0, 2, 1).astype(np.float64)
    return y.astype(np.float32)
